# revision 1
# baseline (speedup 1.0000x reference)
"""LRFGraphConv Trainium2 kernel.

Math: for each vertex i with neighbors N(i) (directed edge list, src=center):
    out[i] = ((sum_{j in N(i)} verts[j] - deg_i * verts[i]) @ lrf[i]) @ W.T + maxN * b

The neighbor-sum commutes with the per-center rotation and GEMM, so the
per-edge work collapses to a segment-sum of neighbor coordinates.  The
rotation and GEMM fuse into a single tensor-engine contraction over the 9
(j,k) pairs of u[i,(j,k)] = t[i,j]*lrf[i,j,k] against Wrep[(j,k),n] = W[n,k],
plus a constant-1 row carrying the maxN*b bias.

Sharding: vertices are partitioned contiguously across 8 cores (6250 each).
The host buckets directed edges by owner of src, builds a per-core padded
neighbor table (maxN slots, zero padded), and gathers the halo neighbor
coordinates into it (the "halo exchange" done at shard time).  Each core runs
the same NEFF on its own shard, processing 7 pipelined chunks of 7 vertex
tiles: DMA in -> DVE strided reduce -> elementwise -> PE transpose -> PE GEMM
(bf16) -> drain -> DMA out.  No collectives.
"""

import os
import sys

sys.path.insert(0, "/opt/trn_rl_repo")

import numpy as np
import ml_dtypes

import concourse.bass as bass
import concourse.bacc as bacc
import concourse.tile as tile
from concourse import mybir
from concourse.masks import make_identity
from concourse.bass_utils import run_bass_kernel_spmd

V = 50000
NCORES = 8
VC = V // NCORES          # 6250 owned vertices per core
P = 128
NVT = (VC + P - 1) // P   # 49 vertex tiles per core
VCP = NVT * P             # 6272 padded
def make_chunks(nbt):
    """Taper: small first chunk (fast pipeline fill), tier-B tiles as their own
    small last chunk (overflow reduce runs when DVE is idle)."""
    rem = NVT - nbt
    ch = [4]
    rem -= 4
    while rem > 8:
        ch.append(8)
        rem -= 8
    if rem:
        ch.append(rem)
    if nbt:
        ch.append(nbt)
    assert sum(ch) == NVT and all(1 <= x <= 8 for x in ch)
    return ch

BF = mybir.dt.float16
BF_NP = np.float16

LAST_RESULTS = None       # BassKernelResults of the most recent run (for test.py)


def build(nc: bass.Bass, NP: int, NBT: int, NPB: int):
    dt = mybir.dt
    xp = nc.dram_tensor("xp", [P, NVT * 3 * NP], BF, kind="ExternalInput")
    xpb = (
        nc.dram_tensor("xpb", [P, NBT * 3 * NPB], BF, kind="ExternalInput")
        if NBT > 0
        else None
    )
    aux = nc.dram_tensor("aux", [P, NVT * 9], BF, kind="ExternalInput")
    wr = nc.dram_tensor("wr", [P, 512], BF, kind="ExternalInput")
    out = nc.dram_tensor("out", [P, NVT * P], dt.float16, kind="ExternalOutput")


    with tile.TileContext(nc) as tc:
        with (
            tc.tile_pool(name="c", bufs=1) as cpool,
            tc.tile_pool(name="x", bufs=3) as xpool,
            tc.tile_pool(name="w", bufs=3) as wpool,
            tc.tile_pool(name="pt", bufs=3, space="PSUM") as pst,
            tc.tile_pool(name="pg", bufs=2, space="PSUM") as psg,
        ):
            ident = cpool.tile([P, P], BF)
            make_identity(nc, ident[:])
            w_t = cpool.tile([P, 512], BF)
            nc.scalar.dma_start(out=w_t[:], in_=wr[:])
            aux_t = cpool.tile([P, NVT * 9], BF)
            nc.scalar.dma_start(out=aux_t[:], in_=aux[:])
            aux9 = aux_t[:].rearrange("p (v f) -> p v f", f=9)
            outsb = cpool.tile([P, NVT * P], dt.float16)
            # persistent u tiles (3-deep rotation); bias slot 9 = 1, 10-15 = 0
            u_bufs = []
            for s in range(3):
                ub = cpool.tile([P, 8 * 16], BF, tag=f"u{s}")
                nc.vector.memset(ub[:], 0.0)
                nc.vector.memset(
                    ub[:].rearrange("p (v s) -> p v s", s=16)[:, :, 9:10], 1.0
                )
                u_bufs.append(ub)

            def _drain_store(p):
                tiles, _olo, _ow = p
                for pg, dsto, ng, hi in tiles:
                    if hi == 0:
                        nc.vector.tensor_copy(
                            out=outsb[:, dsto : dsto + ng * P], in_=pg[:, : ng * P]
                        )
                    else:
                        nc.scalar.copy(
                            out=outsb[:, dsto : dsto + ng * P], in_=pg[:, : ng * P]
                        )
                nc.scalar.dma_start(
                    out=out[:, _olo : _olo + _ow], in_=outsb[:, _olo : _olo + _ow]
                )

            CHUNKS = make_chunks(NBT)
            pending = None
            vlo = 0
            for c, nv in enumerate(CHUNKS):
                cw = nv * 16
                ow = nv * P
                olo = vlo * P
                # load + reduce neighbor sums for this chunk
                xt = xpool.tile([P, 8 * 3 * NP], BF, tag="xt")
                nc.sync.dma_start(
                    out=xt[:, : nv * 3 * NP],
                    in_=xp[:, vlo * 3 * NP : (vlo + nv) * 3 * NP],
                )
                xv = xt[:, : nv * 3 * NP].rearrange(
                    "p (v c n) -> p v c n", v=nv, c=3, n=NP
                )
                # t = sum over slots (one slot holds -deg*verts)
                t = wpool.tile([P, 8 * 3], BF, tag="t")
                with nc.allow_low_precision(reason="fp16 neighbor sums"):
                    nc.vector.tensor_reduce(
                        out=t[:, : nv * 3], in_=xv, axis=mybir.AxisListType.X,
                        op=mybir.AluOpType.add,
                    )
                if NBT > 0 and c == len(CHUNKS) - 1:
                    # overflow slots of high-degree verts (the last NBT v-tiles)
                    xb = cpool.tile([P, NBT * 3 * NPB], BF, tag="xb")
                    nc.sync.dma_start(out=xb[:], in_=xpb[:])
                    tB = cpool.tile([P, NBT * 3], BF, tag="tB")
                    with nc.allow_low_precision(reason="fp16 neighbor sums"):
                        nc.vector.tensor_reduce(
                            out=tB[:],
                            in_=xb[:].rearrange(
                                "p (v c n) -> p v c n", v=NBT, c=3, n=NPB
                            ),
                            axis=mybir.AxisListType.X,
                            op=mybir.AluOpType.add,
                        )
                    nc.vector.tensor_tensor(
                        out=t[:, : nv * 3],
                        in0=t[:, : nv * 3],
                        in1=tB[:],
                        op=mybir.AluOpType.add,
                    )

                # u[p, v, j*3+k] = t[p,v,j]*lrf[p,v,j*3+k] in one broadcast mul
                u = u_bufs[c % 3]
                u9 = u[:, : cw].rearrange("p (v s) -> p v s", s=16)[
                    :, :, 0:9
                ].rearrange("p v (k j) -> p v k j", k=3, j=3)
                t4 = t[:, : nv * 3].rearrange("p (v c) -> p v c", c=3).unsqueeze(2)
                nc.vector.tensor_tensor(
                    out=u9,
                    in0=t4.to_broadcast([P, nv, 3, 3]),
                    in1=aux9[:, vlo : vlo + nv, :].rearrange(
                        "p v (k j) -> p v k j", k=3, j=3
                    ),
                    op=mybir.AluOpType.mult,
                )

                # transpose u [128, nv*16] -> uT [nv*16, 128]
                pt = pst.tile([P, P], BF, tag="pt")
                nc.tensor.transpose(
                    out=pt[:cw, :], in_=u[:, :cw], identity=ident[:]
                )
                uT = wpool.tile([P, P], BF, tag="uT")
                nc.scalar.copy(out=uT[:cw, :], in_=pt[:cw, :])

                # GEMM halves: groups 0-3 (rows 0:64) and 4.. (rows 64:).
                # Each half gets its own PSUM bank and its own drain engine so
                # the two drains run in parallel.
                halves = [(0, min(4, nv))]
                if nv > 4:
                    halves.append((64, nv - 4))
                cur = []
                for hi, (rb, ng) in enumerate(halves):
                    pg = psg.tile([P, 4 * P], dt.float32, tag=f"pg{hi}")
                    nc.tensor.matmul(
                        out=pg[:, : ng * P],
                        lhsT=uT[rb : rb + 16 * ng, :],
                        rhs=w_t[rb : rb + 16 * ng, : ng * P],
                        start=True,
                        stop=True,
                    )
                    cur.append((pg, olo + (rb // 16) * P, ng, (c + hi) % 2))

                # drain + store the PREVIOUS chunk now: in each engine's
                # in-order stream the drain then sits after this chunk's
                # reduce/mul, so waiting on the GEMM no longer stalls them.
                if pending is not None:
                    _drain_store(pending)
                pending = (cur, olo, ow)
                vlo += nv
            _drain_store(pending)
    return nc


def _host_prep(verts, edges, lrf, W, b):
    vb = np.asarray(verts, dtype=np.float32)
    e = np.asarray(edges).astype(np.int64)
    src = np.concatenate([e[:, 0], e[:, 1]]).astype(np.int64)
    dst = np.concatenate([e[:, 1], e[:, 0]]).astype(np.int64)

    deg = np.bincount(src, minlength=V).astype(np.int64)
    maxN = int(deg.max())
    # two-tier: main table has NP slots (last = fold); deg > NP-1 vertices are
    # remapped to the leading v-tiles and spill into the overflow table.
    NP = 24
    CAP = NP - 1
    over = (deg > CAP).reshape(NCORES, VC)
    nB = over.sum(axis=1)
    NBT = int(np.ceil(nB.max() / P)) if maxN > CAP else 0
    NPB = max(0, ((maxN - CAP + 3) // 4) * 4)

    # per-core remap: overflow verts first (stable), then the rest
    newpos = np.empty((NCORES, VC), np.int64)
    order_c = np.empty((NCORES, VC), np.int64)
    for cc in range(NCORES):
        oc = np.concatenate([np.where(~over[cc])[0], np.where(over[cc])[0]])
        order_c[cc] = oc
        newpos[cc, oc] = np.arange(VC)

    order = np.argsort(src, kind="stable")
    src_s = src[order]
    dst_s = dst[order]
    starts = np.zeros(V + 1, np.int64)
    np.cumsum(deg, out=starts[1:])
    slot = np.arange(src_s.size, dtype=np.int64) - starts[src_s]

    c_a = src_s // VC
    il_new = newpos[c_a, src_s - c_a * VC]
    p_a = il_new % P
    v_a = il_new // P
    vals = vb[dst_s].astype(BF_NP)

    Xp = np.zeros((NCORES, P, NVT, 3, NP), BF_NP)
    inA = slot < CAP
    Xp[c_a[inA], p_a[inA], v_a[inA], :, slot[inA]] = vals[inA]
    if NBT > 0:
        XpB = np.zeros((NCORES, P, NBT, 3, NPB), BF_NP)
        inB = ~inA
        XpB[c_a[inB], p_a[inB], v_a[inB] - (NVT - NBT), :, slot[inB] - CAP] = (
            vals[inB]
        )
    else:
        XpB = np.zeros((NCORES, P, 0, 3, 0), BF_NP)

    # fold slot: -deg*verts for the owned vertex goes in the last A slot
    dv = (-deg[:, None].astype(np.float32)) * vb
    dv_pad = np.zeros((NCORES, VCP, 3), np.float32)
    for cc in range(NCORES):
        dv_pad[cc, :VC] = dv.reshape(NCORES, VC, 3)[cc][order_c[cc]]
    Xp[:, :, :, :, NP - 1] = dv_pad.reshape(NCORES, NVT, P, 3).transpose(
        0, 2, 1, 3
    ).astype(BF_NP)

    # aux per vertex: lrf(9), remapped -> [NC, P, NVT*9]
    aux_flat = np.zeros((NCORES, VCP, 9), np.float32)
    # k-major flattening: slot s = k*3+j holds lrf[:, j, k]
    lrf9 = np.ascontiguousarray(
        np.asarray(lrf, np.float32).reshape(NCORES, VC, 3, 3).transpose(0, 1, 3, 2)
    ).reshape(NCORES, VC, 9)
    for cc in range(NCORES):
        aux_flat[cc, :VC] = lrf9[cc][order_c[cc]]
    auxh = np.ascontiguousarray(
        aux_flat.reshape(NCORES, NVT, P, 9).transpose(0, 2, 1, 3)
    ).reshape(NCORES, P, NVT * 9).astype(BF_NP)

    Wf = np.asarray(W, np.float32)
    W16 = np.zeros((16, P), np.float32)
    for s in range(9):
        W16[s, :] = Wf[:, s // 3]   # k-major: slot s = k*3+j -> k = s//3
    W16[9, :] = maxN * np.asarray(b, np.float32)
    # Block-diagonal [128, 512]: 4 column blocks of W16, replicated in both
    # 64-row halves so matmuls can anchor at base partition 0 or 64.
    half = np.zeros((64, 512), np.float32)
    for q in range(4):
        half[16 * q : 16 * q + 16, 128 * q : 128 * q + 128] = W16
    Wr = np.ascontiguousarray(np.vstack([half, half])).astype(BF_NP)

    in_maps = []
    for c in range(NCORES):
        m = {
            "xp": np.ascontiguousarray(Xp[c].reshape(P, NVT * 3 * NP)),
            "aux": np.ascontiguousarray(auxh[c]),
            "wr": Wr,
        }
        if NBT > 0:
            m["xpb"] = np.ascontiguousarray(XpB[c].reshape(P, NBT * 3 * NPB))
        in_maps.append(m)
    return in_maps, NP, NBT, NPB, order_c


def kernel(verts, edges, lrf, W, b):
    global LAST_RESULTS
    in_maps, NP, NBT, NPB, order_c = _host_prep(verts, edges, lrf, W, b)

    nc = bacc.Bacc()
    build(nc, NP, NBT, NPB)
    nc.finalize()

    trace = os.environ.get("KBENCH_TRACE") == "1"
    res = run_bass_kernel_spmd(
        nc, in_maps, core_ids=list(range(NCORES)), trace=trace
    )
    LAST_RESULTS = res

    full = np.empty((V, 128), np.float32)
    for c in range(NCORES):
        o = (
            res.results[c]["out"].astype(np.float32)
            .reshape(P, NVT, P).transpose(1, 0, 2).reshape(VCP, P)[:VC]
        )
        blk = full[c * VC : (c + 1) * VC]
        blk[order_c[c]] = o
    return full

